# revision 7
# baseline (speedup 1.0000x reference)
"""AttractorDynamics Trainium2 kernel.

Reference computation (B=16384, M=1024, A=512, tau=0.1, 10 settling steps):
    drive = x @ W_in.T + b_in                     # [B, A]
    sigma = 0
    repeat 10: sigma = tanh(drive + (sigma @ J.T) / tau)

Strategy: data-parallel over batch across 8 NeuronCores (2048 rows each).
Everything is kept in a transposed [feature, batch] layout so every matmul
has the contraction dim on partitions and no transposes are needed:
    drive^T[a, b] = sum_m W_in^T[m, a] * x^T[m, b]
    (sigma @ Js^T)^T[a', b] = sum_a Js^T[a, a'] * sigma^T[a, b]
with Js = J / tau folded on the host. Matmul operands are bf16 (full-rate
PE, fast weight loads) with fp32 PSUM accumulation; drive is kept fp32 and
the final settling step is computed and stored in fp32.

Per core: 4 batch tiles of 512 columns, processed in interleaved pairs so
the tensor engine always has an independent tile's matmuls to run while
the other tile's add+tanh tail executes.
"""

import numpy as np

B, M, A = 16384, 1024, 512
TAU = 0.1
STEPS = 10
NCORES = 8
BSH = B // NCORES  # 2048 batch rows per core
NB = 512  # batch-tile width (matmul free dim / one PSUM bank)
NBT = BSH // NB  # 4 batch tiles per core
KM = M // 128  # 8 contraction chunks for the input projection
KA = A // 128  # 4 contraction chunks for the recurrence

_CACHE = {}


def _build_nc():
    import sys

    for p in ("/opt/trn_rl_repo",):
        if p not in sys.path:
            sys.path.append(p)
    import concourse.tile as tile
    from concourse import bacc, mybir

    f32 = mybir.dt.float32
    bf16 = mybir.dt.bfloat16
    Tanh = mybir.ActivationFunctionType.Tanh

    nc = bacc.Bacc(None)
    xT = nc.dram_tensor("xT", [M, BSH], bf16, kind="ExternalInput")
    wT = nc.dram_tensor("wT", [M, A], bf16, kind="ExternalInput")
    js = nc.dram_tensor("js", [A, A], bf16, kind="ExternalInput")
    bi = nc.dram_tensor("bi", [A, 1], f32, kind="ExternalInput")
    sigT = nc.dram_tensor("sigT", [A, BSH], f32, kind="ExternalOutput")

    xT_r = xT.rearrange("(k p) n -> p k n", p=128)
    sigT_r = sigT.rearrange("(k p) n -> p k n", p=128)

    with (
        tile.TileContext(nc) as tc,
        tc.tile_pool(name="consts", bufs=1) as consts,
        tc.tile_pool(name="xp", bufs=3) as xp,
        tc.tile_pool(name="dp", bufs=4) as dp,
        tc.tile_pool(name="sp", bufs=6) as sp,
        tc.tile_pool(name="fp", bufs=4) as fp,
        tc.tile_pool(name="psp", bufs=8, space="PSUM") as psp,
    ):
        w_t = consts.tile([128, KM, A], bf16)
        nc.sync.dma_start(out=w_t, in_=wT.rearrange("(k p) a -> p k a", p=128))
        j_t = consts.tile([128, KA, A], bf16)
        nc.sync.dma_start(out=j_t, in_=js.rearrange("(k p) a -> p k a", p=128))
        b_t = consts.tile([128, KA, 1], f32)
        nc.sync.dma_start(out=b_t, in_=bi.rearrange("(k p) o -> p k o", p=128))

        def load_x(bt):
            x_t = xp.tile([128, KM, NB], bf16, tag="x_t")
            nc.sync.dma_start(out=x_t, in_=xT_r[:, :, bt * NB : (bt + 1) * NB])
            return x_t

        def drive_and_sig1(x_t):
            # drive = x @ W_in.T + b (kept for all steps); sigma_1 = tanh(drive)
            d_t = dp.tile([128, KA, NB], f32, tag="d_t")
            s_t = sp.tile([128, KA, NB], bf16, tag="s_t")
            for ma in range(KA):
                ps = psp.tile([128, NB], f32, tag="ps")
                for k in range(KM):
                    nc.tensor.matmul(
                        ps,
                        lhsT=w_t[:, k, ma * 128 : (ma + 1) * 128],
                        rhs=x_t[:, k, :],
                        start=(k == 0),
                        stop=(k == KM - 1),
                    )
                nc.vector.tensor_copy(d_t[:, ma, :], ps)
                nc.scalar.activation(
                    out=s_t[:, ma, :], in_=ps, func=Tanh, bias=b_t[:, ma, :], scale=1.0
                )
            return d_t, s_t

        def settle_step(d_t, s_t, last):
            if last:
                s_new = fp.tile([128, KA, NB], f32, tag="f_t")
            else:
                s_new = sp.tile([128, KA, NB], bf16, tag="s_t")
            for ma in range(KA):
                ps = psp.tile([128, NB], f32, tag="ps")
                for ka in range(KA):
                    nc.tensor.matmul(
                        ps,
                        lhsT=j_t[:, ka, ma * 128 : (ma + 1) * 128],
                        rhs=s_t[:, ka, :],
                        start=(ka == 0),
                        stop=(ka == KA - 1),
                    )
                nc.vector.tensor_add(s_new[:, ma, :], ps, d_t[:, ma, :])
                nc.scalar.activation(
                    out=s_new[:, ma, :],
                    in_=s_new[:, ma, :],
                    func=Tanh,
                    bias=b_t[:, ma, :],
                    scale=1.0,
                )
            return s_new

        for pair0 in range(0, NBT, 2):
            bts = [pair0, pair0 + 1]
            xs = {bt: load_x(bt) for bt in bts}
            dts, sts = {}, {}
            for bt in bts:
                dts[bt], sts[bt] = drive_and_sig1(xs[bt])
            for step in range(STEPS - 1):
                for bt in bts:
                    sts[bt] = settle_step(dts[bt], sts[bt], last=step == STEPS - 2)
            for bt in bts:
                nc.sync.dma_start(
                    out=sigT_r[:, :, bt * NB : (bt + 1) * NB], in_=sts[bt]
                )

    nc.finalize()
    return nc


def _get_nc():
    if "nc" not in _CACHE:
        _CACHE["nc"] = _build_nc()
    return _CACHE["nc"]


def _run(in_maps, **kwargs):
    import sys

    for p in ("/opt/trn_rl_repo",):
        if p not in sys.path:
            sys.path.append(p)
    from concourse.bass_utils import run_bass_kernel_spmd

    return run_bass_kernel_spmd(_get_nc(), in_maps, list(range(NCORES)), **kwargs)


def _make_in_maps(x, W_in, b_in, J):
    import ml_dtypes

    bf = ml_dtypes.bfloat16
    x = np.asarray(x, dtype=np.float32)
    wT = np.ascontiguousarray(np.asarray(W_in, dtype=np.float32).T).astype(bf)
    js = np.ascontiguousarray(
        np.asarray(J, dtype=np.float32).T / TAU
    ).astype(bf)  # [A, A]
    bi = np.ascontiguousarray(np.asarray(b_in, dtype=np.float32).reshape(A, 1))
    in_maps = []
    for c in range(NCORES):
        xc = np.ascontiguousarray(x[c * BSH : (c + 1) * BSH, :].T).astype(bf)
        in_maps.append({"xT": xc, "wT": wT, "js": js, "bi": bi})
    return in_maps


def _assemble(results):
    sigT = np.concatenate([results[c]["sigT"] for c in range(NCORES)], axis=1)
    return np.ascontiguousarray(sigT.T).astype(np.float32)  # [B, A]


def kernel(x, W_in, b_in, J):
    res = _run(_make_in_maps(x, W_in, b_in, J))
    return _assemble(res.results)


# revision 11
# speedup vs baseline: 1.0318x; 1.0318x over previous
"""AttractorDynamics Trainium2 kernel.

Reference computation (B=16384, M=1024, A=512, tau=0.1, 10 settling steps):
    drive = x @ W_in.T + b_in                     # [B, A]
    sigma = 0
    repeat 10: sigma = tanh(drive + (sigma @ J.T) / tau)

Strategy: data-parallel over batch across 8 NeuronCores (2048 rows each).
Everything is kept in a transposed [feature, batch] layout so every matmul
has the contraction dim on partitions and no transposes are needed:
    drive^T[a, b] = sum_m W_in^T[m, a] * x^T[m, b]
    (sigma @ Js^T)^T[a', b] = sum_a Js^T[a, a'] * sigma^T[a, b]
with Js = J / tau folded on the host. Matmul operands are bf16 (full-rate
PE, fast weight loads) with fp32 PSUM accumulation; drive is kept fp32 and
the final settling step is computed and stored in fp32.

Per core: 4 batch tiles of 512 columns, processed in interleaved pairs so
the tensor engine always has an independent tile's matmuls to run while
the other tile's add+tanh tail executes.
"""

import numpy as np

B, M, A = 16384, 1024, 512
TAU = 0.1
STEPS = 10
NCORES = 8
BSH = B // NCORES  # 2048 batch rows per core
NB = 512  # batch-tile width (matmul free dim / one PSUM bank)
NBT = BSH // NB  # 4 batch tiles per core
KM = M // 128  # 8 contraction chunks for the input projection
KA = A // 128  # 4 contraction chunks for the recurrence

_CACHE = {}


def _build_nc():
    import sys

    for p in ("/opt/trn_rl_repo",):
        if p not in sys.path:
            sys.path.append(p)
    import concourse.tile as tile
    from concourse import bacc, mybir

    f32 = mybir.dt.float32
    bf16 = mybir.dt.bfloat16
    Tanh = mybir.ActivationFunctionType.Tanh

    nc = bacc.Bacc(None)
    xT = nc.dram_tensor("xT", [M, BSH], bf16, kind="ExternalInput")
    wT = nc.dram_tensor("wT", [M, A], bf16, kind="ExternalInput")
    js = nc.dram_tensor("js", [A, A], bf16, kind="ExternalInput")
    bi = nc.dram_tensor("bi", [A, 1], f32, kind="ExternalInput")
    sigT = nc.dram_tensor("sigT", [A, BSH], f32, kind="ExternalOutput")

    xT_r = xT.rearrange("(k p) n -> p k n", p=128)
    sigT_r = sigT.rearrange("(k p) n -> p k n", p=128)

    with (
        tile.TileContext(nc) as tc,
        tc.tile_pool(name="consts", bufs=1) as consts,
        tc.tile_pool(name="xp", bufs=3) as xp,
        tc.tile_pool(name="dp", bufs=4) as dp,
        tc.tile_pool(name="sp", bufs=6) as sp,
        tc.tile_pool(name="fp", bufs=4) as fp,
        tc.tile_pool(name="psp", bufs=8, space="PSUM") as psp,
    ):
        wT_r = wT.rearrange("(k p) a -> p k a", p=128)
        w_t = consts.tile([128, KM, A], bf16)
        j_t = consts.tile([128, KA, A], bf16)
        b_t = consts.tile([128, KA, 1], f32)

        def load_x(bt, split=2):
            # Split large loads across DMA queues so transfers overlap.
            x_t = xp.tile([128, KM, NB], bf16, tag="x_t")
            step = KM // split
            for h in range(split):
                nc.sync.dma_start(
                    out=x_t[:, h * step : (h + 1) * step, :],
                    in_=xT_r[:, h * step : (h + 1) * step, bt * NB : (bt + 1) * NB],
                )
            return x_t

        def load_startup():
            # Interleave the first x tile's halves with W's halves so the
            # transfers that gate the first matmuls are issued first and run
            # on parallel queues; j/b are needed a few microseconds later.
            x0 = xp.tile([128, KM, NB], bf16, tag="x_t")
            for h in range(2):
                nc.sync.dma_start(
                    out=x0[:, h * 4 : (h + 1) * 4, :],
                    in_=xT_r[:, h * 4 : (h + 1) * 4, 0:NB],
                )
                nc.sync.dma_start(
                    out=w_t[:, h * 4 : (h + 1) * 4, :],
                    in_=wT_r[:, h * 4 : (h + 1) * 4, :],
                )
            nc.sync.dma_start(out=j_t, in_=js.rearrange("(k p) a -> p k a", p=128))
            nc.sync.dma_start(out=b_t, in_=bi.rearrange("(k p) o -> p k o", p=128))
            return x0

        def drive_and_sig1(x_t):
            # drive = x @ W_in.T + b (kept for all steps); sigma_1 = tanh(drive)
            d_t = dp.tile([128, KA, NB], f32, tag="d_t")
            s_t = sp.tile([128, KA, NB], bf16, tag="s_t")
            for ma in range(KA):
                ps = psp.tile([128, NB], f32, tag="ps")
                for k in range(KM):
                    nc.tensor.matmul(
                        ps,
                        lhsT=w_t[:, k, ma * 128 : (ma + 1) * 128],
                        rhs=x_t[:, k, :],
                        start=(k == 0),
                        stop=(k == KM - 1),
                    )
                nc.vector.tensor_copy(d_t[:, ma, :], ps)
                nc.scalar.activation(
                    out=s_t[:, ma, :], in_=ps, func=Tanh, bias=b_t[:, ma, :], scale=1.0
                )
            return d_t, s_t

        def settle_step(d_t, s_t, bt, last):
            if last:
                s_new = fp.tile([128, KA, NB], f32, tag="f_t")
            else:
                s_new = sp.tile([128, KA, NB], bf16, tag="s_t")
            for ma in range(KA):
                ps = psp.tile([128, NB], f32, tag="ps")
                for ka in range(KA):
                    nc.tensor.matmul(
                        ps,
                        lhsT=j_t[:, ka, ma * 128 : (ma + 1) * 128],
                        rhs=s_t[:, ka, :],
                        start=(ka == 0),
                        stop=(ka == KA - 1),
                    )
                nc.vector.tensor_add(s_new[:, ma, :], ps, d_t[:, ma, :])
                nc.scalar.activation(
                    out=s_new[:, ma, :],
                    in_=s_new[:, ma, :],
                    func=Tanh,
                    bias=b_t[:, ma, :],
                    scale=1.0,
                )
                if last:
                    # Stream each finished chunk out on the idle GpSimd queue
                    # so the store overlaps the remaining compute.
                    nc.gpsimd.dma_start(
                        out=sigT_r[:, ma, bt * NB : (bt + 1) * NB],
                        in_=s_new[:, ma, :],
                    )
            return s_new

        x_pre = {0: load_startup()}
        for pair0 in range(0, NBT, 2):
            bts = [pair0, pair0 + 1]
            xs = {bt: x_pre.pop(bt) if bt in x_pre else load_x(bt) for bt in bts}
            dts, sts = {}, {}
            for bt in bts:
                dts[bt], sts[bt] = drive_and_sig1(xs[bt])
            for step in range(STEPS - 1):
                for bt in bts:
                    sts[bt] = settle_step(
                        dts[bt], sts[bt], bt, last=step == STEPS - 2
                    )

    nc.finalize()
    return nc


def _get_nc():
    if "nc" not in _CACHE:
        _CACHE["nc"] = _build_nc()
    return _CACHE["nc"]


def _run(in_maps, **kwargs):
    import sys

    for p in ("/opt/trn_rl_repo",):
        if p not in sys.path:
            sys.path.append(p)
    from concourse.bass_utils import run_bass_kernel_spmd

    return run_bass_kernel_spmd(_get_nc(), in_maps, list(range(NCORES)), **kwargs)


def _make_in_maps(x, W_in, b_in, J):
    import ml_dtypes

    bf = ml_dtypes.bfloat16
    x = np.asarray(x, dtype=np.float32)
    wT = np.ascontiguousarray(np.asarray(W_in, dtype=np.float32).T).astype(bf)
    js = np.ascontiguousarray(
        np.asarray(J, dtype=np.float32).T / TAU
    ).astype(bf)  # [A, A]
    bi = np.ascontiguousarray(np.asarray(b_in, dtype=np.float32).reshape(A, 1))
    in_maps = []
    for c in range(NCORES):
        xc = np.ascontiguousarray(x[c * BSH : (c + 1) * BSH, :].T).astype(bf)
        in_maps.append({"xT": xc, "wT": wT, "js": js, "bi": bi})
    return in_maps


def _assemble(results):
    sigT = np.concatenate([results[c]["sigT"] for c in range(NCORES)], axis=1)
    return np.ascontiguousarray(sigT.T).astype(np.float32)  # [B, A]


def kernel(x, W_in, b_in, J):
    res = _run(_make_in_maps(x, W_in, b_in, J))
    return _assemble(res.results)


# revision 12
# speedup vs baseline: 1.0404x; 1.0083x over previous
"""AttractorDynamics Trainium2 kernel.

Reference computation (B=16384, M=1024, A=512, tau=0.1, 10 settling steps):
    drive = x @ W_in.T + b_in                     # [B, A]
    sigma = 0
    repeat 10: sigma = tanh(drive + (sigma @ J.T) / tau)

Strategy: data-parallel over batch across 8 NeuronCores (2048 rows each).
Everything is kept in a transposed [feature, batch] layout so every matmul
has the contraction dim on partitions and no transposes are needed:
    drive^T[a, b] = sum_m W_in^T[m, a] * x^T[m, b]
    (sigma @ Js^T)^T[a', b] = sum_a Js^T[a, a'] * sigma^T[a, b]
with Js = J / tau folded on the host. Matmul operands are bf16 (full-rate
PE, fast weight loads) with fp32 PSUM accumulation; drive is kept fp32 and
the final settling step is computed and stored in fp32.

Per core the 2048 batch columns are cut into tiles [256, 512, 512, 512,
256] processed as two interleaved groups (3 then 2 tiles) so the tensor
engine always has another tile's matmuls while a tile's add+tanh tail
runs. The narrow first tile gets the first matmul started sooner after the
DMA preamble; the narrow last tile shortens the final drain. Output chunks
stream out on alternating DMA queues as soon as their final tanh lands.
"""

import numpy as np

B, M, A = 16384, 1024, 512
TAU = 0.1
STEPS = 10
NCORES = 8
BSH = B // NCORES  # 2048 batch rows per core
KM = M // 128  # 8 contraction chunks for the input projection
KA = A // 128  # 4 contraction chunks for the recurrence

# (column offset, width) per batch tile; groups are processed sequentially
# with round-robin settling inside each group.
TILES = [(0, 256), (256, 512), (768, 512), (1280, 512), (1792, 256)]
GROUPS = [(0, 1, 2), (3, 4)]

_CACHE = {}


def _build_nc():
    import sys

    for p in ("/opt/trn_rl_repo",):
        if p not in sys.path:
            sys.path.append(p)
    import concourse.tile as tile
    from concourse import bacc, mybir

    f32 = mybir.dt.float32
    bf16 = mybir.dt.bfloat16
    Tanh = mybir.ActivationFunctionType.Tanh

    nc = bacc.Bacc(None)
    xT = nc.dram_tensor("xT", [M, BSH], bf16, kind="ExternalInput")
    wT = nc.dram_tensor("wT", [M, A], bf16, kind="ExternalInput")
    js = nc.dram_tensor("js", [A, A], bf16, kind="ExternalInput")
    bi = nc.dram_tensor("bi", [A, 1], f32, kind="ExternalInput")
    sigT = nc.dram_tensor("sigT", [A, BSH], f32, kind="ExternalOutput")

    xT_r = xT.rearrange("(k p) n -> p k n", p=128)
    wT_r = wT.rearrange("(k p) a -> p k a", p=128)
    sigT_r = sigT.rearrange("(k p) n -> p k n", p=128)

    with (
        tile.TileContext(nc) as tc,
        tc.tile_pool(name="consts", bufs=1) as consts,
        tc.tile_pool(name="xp", bufs=5) as xp,
        tc.tile_pool(name="dp", bufs=5) as dp,
        tc.tile_pool(name="sp", bufs=8) as sp,
        tc.tile_pool(name="fp", bufs=3) as fp,
        tc.tile_pool(name="psp", bufs=8, space="PSUM") as psp,
    ):
        w_t = consts.tile([128, KM, A], bf16)
        j_t = consts.tile([128, KA, A], bf16)
        b_t = consts.tile([128, KA, 1], f32)

        def load_x(ti, nsplit):
            off, w = TILES[ti]
            x_t = xp.tile([128, KM, w], bf16, tag="x_t")
            step = KM // nsplit
            for h in range(nsplit):
                nc.sync.dma_start(
                    out=x_t[:, h * step : (h + 1) * step, :],
                    in_=xT_r[:, h * step : (h + 1) * step, off : off + w],
                )
            return x_t

        def drive_and_sig1(ti, x_t):
            # drive = x @ W_in.T + b (kept for all steps); sigma_1 = tanh(drive)
            off, w = TILES[ti]
            d_t = dp.tile([128, KA, w], f32, tag="d_t")
            s_t = sp.tile([128, KA, w], bf16, tag="s_t")
            for ma in range(KA):
                ps = psp.tile([128, w], f32, tag="ps")
                for k in range(KM):
                    nc.tensor.matmul(
                        ps,
                        lhsT=w_t[:, k, ma * 128 : (ma + 1) * 128],
                        rhs=x_t[:, k, :],
                        start=(k == 0),
                        stop=(k == KM - 1),
                    )
                nc.vector.tensor_copy(d_t[:, ma, :], ps)
                nc.scalar.activation(
                    out=s_t[:, ma, :], in_=ps, func=Tanh, bias=b_t[:, ma, :], scale=1.0
                )
            return d_t, s_t

        def settle_step(ti, d_t, s_t, last):
            off, w = TILES[ti]
            if last:
                s_new = fp.tile([128, KA, w], f32, tag="f_t")
            else:
                s_new = sp.tile([128, KA, w], bf16, tag="s_t")
            for ma in range(KA):
                ps = psp.tile([128, w], f32, tag="ps")
                for ka in range(KA):
                    nc.tensor.matmul(
                        ps,
                        lhsT=j_t[:, ka, ma * 128 : (ma + 1) * 128],
                        rhs=s_t[:, ka, :],
                        start=(ka == 0),
                        stop=(ka == KA - 1),
                    )
                nc.vector.tensor_add(s_new[:, ma, :], ps, d_t[:, ma, :])
                nc.scalar.activation(
                    out=s_new[:, ma, :],
                    in_=s_new[:, ma, :],
                    func=Tanh,
                    bias=b_t[:, ma, :],
                    scale=1.0,
                )
                if last:
                    # Stream each finished chunk out immediately, alternating
                    # DMA queues so the flush overlaps remaining compute.
                    eng = nc.gpsimd if ma % 2 == 0 else nc.sync
                    eng.dma_start(
                        out=sigT_r[:, ma, off : off + w], in_=s_new[:, ma, :]
                    )
            return s_new

        # Startup: the first (narrow) x tile and the first W quarter gate the
        # first matmuls — issue them first, on parallel queues. j/b are not
        # needed until the first settling step.
        xs = {0: load_x(0, 1)}
        for q in range(4):
            nc.sync.dma_start(
                out=w_t[:, q * 2 : (q + 1) * 2, :],
                in_=wT_r[:, q * 2 : (q + 1) * 2, :],
            )
        xs[1] = load_x(1, 2)
        nc.sync.dma_start(out=j_t, in_=js.rearrange("(k p) a -> p k a", p=128))
        nc.sync.dma_start(out=b_t, in_=bi.rearrange("(k p) o -> p k o", p=128))
        xs[2] = load_x(2, 2)

        for gi, group in enumerate(GROUPS):
            dts, sts = {}, {}
            for ti in group:
                dts[ti], sts[ti] = drive_and_sig1(ti, xs.pop(ti))
            if gi + 1 < len(GROUPS):
                for ti in GROUPS[gi + 1]:
                    xs[ti] = load_x(ti, 2)
            for step in range(STEPS - 1):
                for ti in group:
                    sts[ti] = settle_step(
                        ti, dts[ti], sts[ti], last=step == STEPS - 2
                    )

    nc.finalize()
    return nc


def _get_nc():
    if "nc" not in _CACHE:
        _CACHE["nc"] = _build_nc()
    return _CACHE["nc"]


def _run(in_maps, **kwargs):
    import sys

    for p in ("/opt/trn_rl_repo",):
        if p not in sys.path:
            sys.path.append(p)
    from concourse.bass_utils import run_bass_kernel_spmd

    return run_bass_kernel_spmd(_get_nc(), in_maps, list(range(NCORES)), **kwargs)


def _make_in_maps(x, W_in, b_in, J):
    import ml_dtypes

    bf = ml_dtypes.bfloat16
    x = np.asarray(x, dtype=np.float32)
    wT = np.ascontiguousarray(np.asarray(W_in, dtype=np.float32).T).astype(bf)
    js = np.ascontiguousarray(
        np.asarray(J, dtype=np.float32).T / TAU
    ).astype(bf)  # [A, A]
    bi = np.ascontiguousarray(np.asarray(b_in, dtype=np.float32).reshape(A, 1))
    in_maps = []
    for c in range(NCORES):
        xc = np.ascontiguousarray(x[c * BSH : (c + 1) * BSH, :].T).astype(bf)
        in_maps.append({"xT": xc, "wT": wT, "js": js, "bi": bi})
    return in_maps


def _assemble(results):
    sigT = np.concatenate([results[c]["sigT"] for c in range(NCORES)], axis=1)
    return np.ascontiguousarray(sigT.T).astype(np.float32)  # [B, A]


def kernel(x, W_in, b_in, J):
    res = _run(_make_in_maps(x, W_in, b_in, J))
    return _assemble(res.results)
